# revision 13
# baseline (speedup 1.0000x reference)
"""MHSA over 32 independent 512-token segments, segment-parallel across 8
NeuronCores (4 segments / 2048 tokens per core, zero cross-core traffic).

v2: software-pipelined single-pass design.

Per core, per segment s (tokens [512s, 512s+512)):
  x^T        via PE f32r transposes (identity moving operand, 1.5 cyc/row)
  Q^T,K^T    lhsT = streamed W tile (f32r straight from DMA; the DRAM
             tensors are declared float32r so no conversion copies), rhs =
             x^T chunks; accumulate per 128-col m-tile in one PSUM bank.
  V          natural [tok, 1024] via lhsT = x^T chunk, rhs = W tile -> bf16
  S^T        = K Q^T per head, 4 banks; exp via Act in 2 halves (bf16 out)
  O          natural per (head, qt): lhsT = A^T chunk [128k,128q] bf16,
             rhs = V|1 [128k, 65] bf16 -> PSUM [128q, 65]; 27 ns/matmul.
  normalize  reciprocal of col 64 (strided) + tensor_scalar_mul with the
             [128,1] per-partition scalar -> Y bf16 (no broadcast matmuls)
  Y^T        bf16 PE transposes (1 cyc/row) into a bf16-viewed PSUM bank
  out        = Y^T.T @ Wproj (lhsT bf16, rhs f32r)

Emission is pipelined: during segment s's attention heads (Act-bound), PE
is fed filler units: Y^T + projection of segment s-1 and x^T of segment
s+1. PE stays near 100% busy and never drops out of its high p-state.
"""

import numpy as np

import concourse.bass as bass
import concourse.mybir as mybir
import concourse.tile as tile
from concourse.bass_utils import run_bass_kernel_spmd

F32 = mybir.dt.float32
F32R = mybir.dt.float32r
BF16 = mybir.dt.bfloat16
EXP = mybir.ActivationFunctionType.Exp

T, C, H, HD = 16384, 1024, 16, 64
NCORES = 8
TOK = T // NCORES          # 2048 tokens per core
SEG = 512                  # tokens per segment
NSEG = TOK // SEG          # 4 segments per core
SCALE = 1.0 / np.sqrt(HD)  # folded into exp()


def _split_multi_waits(nc):
    """Move extra sync waits onto same-engine NoOps (1-wait ISA limit)."""
    for fn in nc.m.functions:
        for bb in fn.blocks:
            out = []
            for inst in bb.instructions:
                si = inst.sync_info
                if si is not None and si.on_wait and len(si.on_wait) > 1:
                    waits = list(si.on_wait)
                    for j, w in enumerate(waits[:-1]):
                        nop = mybir.InstNoOp(name=f"{inst.name}-wsp{j}")
                        nop.engine = inst.engine
                        nop.sync_info = mybir.SyncInfo(on_wait=[w], on_update=[])
                        out.append(nop)
                    inst.sync_info = mybir.SyncInfo(
                        on_wait=[waits[-1]], on_update=list(si.on_update)
                    )
                out.append(inst)
            bb.instructions = out


def _build():
    nc = bass.Bass("TRN2", target_bir_lowering=False, debug=False)
    x = nc.dram_tensor("x_sh", [TOK, C], F32R, kind="ExternalInput").ap()
    wa = nc.dram_tensor("w_attn", [C, 3 * C], F32R, kind="ExternalInput").ap()
    wp = nc.dram_tensor("w_proj", [C, C], F32R, kind="ExternalInput").ap()
    out = nc.dram_tensor("out", [TOK, C], F32, kind="ExternalOutput").ap()

    ident_d = nc.inline_tensor(np.eye(128, dtype=np.float32), "ident_c").ap()

    with tile.TileContext(nc) as tc:
        with (
            tc.tile_pool(name="const", bufs=1) as cpool,
            tc.tile_pool(name="wres", bufs=1) as wres,
            tc.tile_pool(name="stream", bufs=1) as stream,
            tc.tile_pool(name="work", bufs=1) as work,
            tc.tile_pool(name="ps", bufs=1, space="PSUM") as pspool,
        ):
            ps = pspool.tile([128, 4096], F32, tag="ps", name="ps")

            idf = cpool.tile([128, 128], F32, tag="idf", name="idf")
            idr = cpool.tile([128, 128], F32R, tag="idr", name="idr")
            idb = cpool.tile([128, 128], BF16, tag="idb", name="idb")
            nc.sync.dma_start(idf[:], ident_d[:, :])
            nc.vector.tensor_copy(idr[:], idf[:])
            nc.vector.tensor_copy(idb[:], idf[:])

            # wproj in bf16: walrus forbids mixing f32r with bf16 matmul
            # operands, and the Y^T side is bf16.
            wproj = [wres.tile([128, C], BF16, tag=f"wp{cc}", name=f"wp{cc}")
                     for cc in range(8)]

            # persistent single-generation big tiles
            xTall = work.tile([128, 4096], F32R, tag="xTall", name="xTall")
            qktall = work.tile([128, 8192], F32R, tag="qktall", name="qktall")
            ytall = work.tile([128, 4096], BF16, tag="ytall", name="ytall")

            # accumulation slot ping-pong over PSUM banks 4,5
            _pp = [0]

            def bank():
                b = 2048 + (_pp[0] % 2) * 512
                _pp[0] += 1
                return ps[0:128, b:b + 512]

            # ---------------- unit emitters ----------------
            def emit_A(snx, qt):
                """x^T for segment snx, token-tile qt -> xTall cols qt*128."""
                xn = stream.tile([128, C], F32R, tag="xn", bufs=3,
                                 name=f"xn{snx}_{qt}")
                nc.sync.dma_start(
                    xn[:], x[snx * SEG + qt * 128: snx * SEG + (qt + 1) * 128, :])
                b0 = 2048
                _pp[0] += 2  # takes both slots
                for j in range(8):
                    nc.tensor.transpose(
                        ps[0:128, b0 + j * 128: b0 + (j + 1) * 128].bitcast(F32R),
                        xn[:, j * 128:(j + 1) * 128], idr[:])
                dst = xTall[:].rearrange("p (c w) -> p c w", w=512)[:, :, qt * 128:(qt + 1) * 128]
                src = ps[0:128, b0:b0 + 1024].rearrange("p (c w) -> p c w", w=128)
                nc.vector.tensor_copy(dst, src)

            def emit_Bdma(s, grp):
                """grp 0-3: QK (g=grp//2, half=grp%2); grp 4-5: V (vn=grp-4).
                All six are 8 DMAs of [128,512] w_attn column panels."""
                off = grp * 512
                tiles = []
                for cc in range(8):
                    w = stream.tile([128, 512], F32R, tag="ws", bufs=20,
                                    name=f"w{s}_{grp}_{cc}")
                    nc.sync.dma_start(
                        w[:], wa[cc * 128:(cc + 1) * 128, off: off + 512])
                    tiles.append(w)
                return tiles

            def emit_Bqk_unit(tiles, g, half, m):
                b = bank()
                for cc in range(8):
                    nc.tensor.matmul(
                        b, tiles[cc][:, m * 128:(m + 1) * 128],
                        xTall[:, cc * 512:(cc + 1) * 512],
                        start=(cc == 0), stop=(cc == 7))
                idx = g * 8 + half * 4 + m
                nc.vector.tensor_copy(qktall[:, idx * 512:(idx + 1) * 512], b)

            def emit_Bv_unit(tiles, vn, qt, vptiles):
                b = bank()
                for cc in range(8):
                    nc.tensor.matmul(
                        b, xTall[:, cc * 512 + qt * 128: cc * 512 + (qt + 1) * 128],
                        tiles[cc][:], start=(cc == 0), stop=(cc == 7))
                nc.vector.tensor_copy(
                    vptiles[qt].rearrange("p (h w) -> p h w", w=66)[:, vn * 8:(vn + 1) * 8, 0:64],
                    b.rearrange("p (h w) -> p h w", w=64))

            def emit_head(s, h, vptiles, Ytiles):
                q0 = (h // 2) * 512
                k0 = (8 + h // 2) * 512
                r0 = (h % 2) * 64
                at = stream.tile([128, 2048], BF16, tag="at0", bufs=3,
                                 name=f"at{s}_{h}")
                for half in range(2):
                    for k2 in range(2):
                        kt = half * 2 + k2
                        nc.tensor.matmul(
                            ps[0:128, kt * 512:(kt + 1) * 512],
                            qktall[r0:r0 + 64, k0 + kt * 128: k0 + (kt + 1) * 128],
                            qktall[r0:r0 + 64, q0:q0 + 512],
                            start=True, stop=True)
                    nc.scalar.activation(
                        at[:, half * 1024:(half + 1) * 1024],
                        ps[0:128, half * 1024:(half + 1) * 1024], EXP, scale=SCALE)
                base = 3072 + (h % 2) * 512
                for qt in range(4):
                    for kt in range(4):
                        nc.tensor.matmul(
                            ps[0:128, base + qt * 65: base + qt * 65 + 65],
                            at[:, kt * 512 + qt * 128: kt * 512 + (qt + 1) * 128],
                            vptiles[kt][:, 66 * h: 66 * h + 65],
                            start=(kt == 0), stop=(kt == 3))
                rz = stream.tile([128, 4], F32, tag="rz", bufs=2, name=f"rz{s}_{h}")
                nc.vector.reciprocal(
                    rz[:],
                    ps[0:128, base:base + 260].rearrange("p (a b) -> p a b", b=65)[:, :, 64:65])
                for qt in range(4):
                    nc.vector.tensor_scalar_mul(
                        Ytiles[qt][:, h * 64:(h + 1) * 64],
                        ps[0:128, base + qt * 65: base + qt * 65 + 64],
                        rz[:, qt:qt + 1])

            def emit_Yt(qt, Yprev, eng=None):
                b = bank()
                psb = b.bitcast(BF16)  # [128, 1024] bf16 = one f32 bank
                for cc in range(8):
                    nc.tensor.transpose(
                        psb[:, cc * 128:(cc + 1) * 128],
                        Yprev[qt][:, cc * 128:(cc + 1) * 128], idb[:])
                dst = ytall[:].rearrange("p (c w) -> p c w", w=512)[:, :, qt * 128:(qt + 1) * 128]
                src = psb.rearrange("p (c w) -> p c w", w=128)
                if eng == "scalar":
                    nc.scalar.copy(dst, src)
                else:
                    nc.vector.tensor_copy(dst, src)

            def emit_D(sp, m, vn, obt):
                b = bank()
                for cc in range(8):
                    nc.tensor.matmul(
                        b, ytall[:, cc * 512 + m * 128: cc * 512 + (m + 1) * 128],
                        wproj[cc][:, vn * 512:(vn + 1) * 512],
                        start=(cc == 0), stop=(cc == 7))
                nc.scalar.copy(obt[:, vn * 512:(vn + 1) * 512], b)
                if vn == 1:
                    nc.sync.dma_start(
                        out[sp * SEG + m * 128: sp * SEG + (m + 1) * 128, :], obt[:])

            # ---------------- prologue ----------------
            for qt in range(4):
                emit_A(0, qt)

            Yprev = None
            # first weight-DMA group of segment 0
            next_tiles = emit_Bdma(0, 0)
            for s in range(NSEG):
                vptiles = [stream.tile([128, 16 * 66], BF16, tag=f"vp{qt}",
                                       bufs=2, name=f"vp{s}_{qt}")
                           for qt in range(4)]
                for qt in range(4):
                    nc.vector.memset(
                        vptiles[qt].rearrange("p (h w) -> p h w", w=66)[:, :, 64:65], 1.0)

                # ---- B: 6 groups, DMA emitted one group ahead of use
                for grp in range(6):
                    tiles = next_tiles
                    if grp < 5:
                        next_tiles = emit_Bdma(s, grp + 1)
                    if grp < 4:
                        g, half = grp // 2, grp % 2
                        for m in range(4):
                            emit_Bqk_unit(tiles, g, half, m)
                    else:
                        vn = grp - 4
                        for qt in range(4):
                            emit_Bv_unit(tiles, vn, qt, vptiles)

                # ---- C: heads with pipelined fillers
                Ytiles = [stream.tile([128, C], BF16, tag=f"Y{qt}", bufs=2,
                                      name=f"Y{s}_{qt}")
                          for qt in range(4)]
                obtiles = [stream.tile([128, C], F32, tag="ob", bufs=2,
                                       name=f"ob{s}_{m}") for m in range(4)]
                fillers = []
                nonlocal_box = [None]
                if s < NSEG - 1:
                    # prefetch next segment's first weight group early
                    def _prefetch(snx):
                        nonlocal_box[0] = emit_Bdma(snx, 0)
                    fillers.append(lambda snx=s + 1: _prefetch(snx))
                if s > 0:
                    for m in range(4):
                        fillers.append(lambda m=m: emit_Yt(m, Yprev))
                    for m in range(4):
                        fillers.append(lambda m=m: emit_D(s - 1, m, 0, obtiles[m]))
                        fillers.append(lambda m=m: emit_D(s - 1, m, 1, obtiles[m]))
                else:
                    def _wpstage(cc):
                        wst = stream.tile([128, C], F32R, tag="xn", bufs=3,
                                          name=f"wpst{cc}")
                        nc.sync.dma_start(wst[:], wp[cc * 128:(cc + 1) * 128, :])
                        nc.gpsimd.tensor_copy(wproj[cc][:], wst[:])
                    for cc in range(8):
                        fillers.append(lambda cc=cc: _wpstage(cc))
                if s < NSEG - 1:
                    for qt in range(4):
                        fillers.append(lambda qt=qt: emit_A(s + 1, qt))
                nf = len(fillers)
                done = 0
                for h in range(16):
                    emit_head(s, h, vptiles, Ytiles)
                    want = (h + 1) * nf // 16
                    while done < want:
                        fillers[done]()
                        done += 1
                if s < NSEG - 1:
                    next_tiles = nonlocal_box[0]
                Yprev = Ytiles

            # ---- epilogue: last segment's projection (Yt first, then D)
            obtiles = [stream.tile([128, C], F32, tag="ob", bufs=2,
                                   name=f"obE_{m}") for m in range(4)]
            for m in range(4):
                emit_Yt(m, Yprev, eng="scalar")
            for m in range(4):
                emit_D(NSEG - 1, m, 0, obtiles[m])
                emit_D(NSEG - 1, m, 1, obtiles[m])

    _split_multi_waits(nc)
    return nc


_NC = None


def kernel(x, w_attn, w_proj, split_sections):
    global _NC
    if _NC is None:
        _NC = _build()
    x = np.ascontiguousarray(np.asarray(x, dtype=np.float32))
    w_attn = np.ascontiguousarray(np.asarray(w_attn, dtype=np.float32))
    w_proj = np.ascontiguousarray(np.asarray(w_proj, dtype=np.float32))
    in_maps = [
        {"x_sh": x[i * TOK:(i + 1) * TOK], "w_attn": w_attn, "w_proj": w_proj}
        for i in range(NCORES)
    ]
    res = run_bass_kernel_spmd(_NC, in_maps, core_ids=list(range(NCORES)))
    return np.concatenate([res.results[i]["out"] for i in range(NCORES)], axis=0)


if __name__ == "__main__":
    rng = np.random.default_rng(0)
    x = rng.standard_normal((T, C), dtype=np.float32)
    wa = (rng.standard_normal((C, 3 * C), dtype=np.float32) / np.sqrt(C)).astype(np.float32)
    wpj = (rng.standard_normal((C, C), dtype=np.float32) / np.sqrt(C)).astype(np.float32)
    y = kernel(x, wa, wpj, np.arange(1, 32) * 512)
    print("out", y.shape, y.dtype, np.abs(y).mean())


# revision 16
# speedup vs baseline: 1.0262x; 1.0262x over previous
"""MHSA over 32 independent 512-token segments, segment-parallel across 8
NeuronCores (4 segments / 2048 tokens per core, zero cross-core traffic).

v2: software-pipelined single-pass design.

Per core, per segment s (tokens [512s, 512s+512)):
  x^T        via PE f32r transposes (identity moving operand, 1.5 cyc/row)
  Q^T,K^T    lhsT = streamed W tile (f32r straight from DMA; the DRAM
             tensors are declared float32r so no conversion copies), rhs =
             x^T chunks; accumulate per 128-col m-tile in one PSUM bank.
  V          natural [tok, 1024] via lhsT = x^T chunk, rhs = W tile -> bf16
  S^T        = K Q^T per head, 4 banks; exp via Act in 2 halves (bf16 out)
  O          natural per (head, qt): lhsT = A^T chunk [128k,128q] bf16,
             rhs = V|1 [128k, 65] bf16 -> PSUM [128q, 65]; 27 ns/matmul.
  normalize  reciprocal of col 64 (strided) + tensor_scalar_mul with the
             [128,1] per-partition scalar -> Y bf16 (no broadcast matmuls)
  Y^T        bf16 PE transposes (1 cyc/row) into a bf16-viewed PSUM bank
  out        = Y^T.T @ Wproj (lhsT bf16, rhs f32r)

Emission is pipelined: during segment s's attention heads (Act-bound), PE
is fed filler units: Y^T + projection of segment s-1 and x^T of segment
s+1. PE stays near 100% busy and never drops out of its high p-state.
"""

import numpy as np

import concourse.bass as bass
import concourse.mybir as mybir
import concourse.tile as tile
from concourse.bass_utils import run_bass_kernel_spmd

F32 = mybir.dt.float32
F32R = mybir.dt.float32r
BF16 = mybir.dt.bfloat16
EXP = mybir.ActivationFunctionType.Exp

T, C, H, HD = 16384, 1024, 16, 64
NCORES = 8
TOK = T // NCORES          # 2048 tokens per core
SEG = 512                  # tokens per segment
NSEG = TOK // SEG          # 4 segments per core
SCALE = 1.0 / np.sqrt(HD)  # folded into exp()


def _split_multi_waits(nc):
    """Move extra sync waits onto same-engine NoOps (1-wait ISA limit)."""
    for fn in nc.m.functions:
        for bb in fn.blocks:
            out = []
            for inst in bb.instructions:
                si = inst.sync_info
                if si is not None and si.on_wait and len(si.on_wait) > 1:
                    waits = list(si.on_wait)
                    for j, w in enumerate(waits[:-1]):
                        nop = mybir.InstNoOp(name=f"{inst.name}-wsp{j}")
                        nop.engine = inst.engine
                        nop.sync_info = mybir.SyncInfo(on_wait=[w], on_update=[])
                        out.append(nop)
                    inst.sync_info = mybir.SyncInfo(
                        on_wait=[waits[-1]], on_update=list(si.on_update)
                    )
                out.append(inst)
            bb.instructions = out


def _build():
    nc = bass.Bass("TRN2", target_bir_lowering=False, debug=False)
    x = nc.dram_tensor("x_sh", [TOK, C], F32R, kind="ExternalInput").ap()
    wa = nc.dram_tensor("w_attn", [C, 3 * C], F32R, kind="ExternalInput").ap()
    wp = nc.dram_tensor("w_proj", [C, C], F32R, kind="ExternalInput").ap()
    out = nc.dram_tensor("out", [TOK, C], F32, kind="ExternalOutput").ap()

    ident_d = nc.inline_tensor(np.eye(128, dtype=np.float32), "ident_c").ap()

    with tile.TileContext(nc) as tc:
        with (
            tc.tile_pool(name="const", bufs=1) as cpool,
            tc.tile_pool(name="wres", bufs=1) as wres,
            tc.tile_pool(name="stream", bufs=1) as stream,
            tc.tile_pool(name="work", bufs=1) as work,
            tc.tile_pool(name="ps", bufs=1, space="PSUM") as pspool,
        ):
            ps = pspool.tile([128, 4096], F32, tag="ps", name="ps")

            idf = cpool.tile([128, 128], F32, tag="idf", name="idf")
            idr = cpool.tile([128, 128], F32R, tag="idr", name="idr")
            idb = cpool.tile([128, 128], BF16, tag="idb", name="idb")
            nc.sync.dma_start(idf[:], ident_d[:, :])
            nc.vector.tensor_copy(idr[:], idf[:])
            nc.vector.tensor_copy(idb[:], idf[:])

            # wproj in bf16: walrus forbids mixing f32r with bf16 matmul
            # operands, and the Y^T side is bf16.
            wproj = [wres.tile([128, C], BF16, tag=f"wp{cc}", name=f"wp{cc}")
                     for cc in range(8)]

            # persistent single-generation big tiles
            xTall = work.tile([128, 4096], F32R, tag="xTall", name="xTall")
            qktall = work.tile([128, 8192], F32R, tag="qktall", name="qktall")
            ytall = work.tile([128, 4096], BF16, tag="ytall", name="ytall")

            # accumulation slot ping-pong over PSUM banks 4,5
            _pp = [0]

            def bank():
                b = 2048 + (_pp[0] % 2) * 512
                _pp[0] += 1
                return ps[0:128, b:b + 512]

            # ---------------- unit emitters ----------------
            def emit_A(snx, qt):
                """x^T for segment snx, token-tile qt -> xTall cols qt*128."""
                xn = stream.tile([128, C], F32R, tag="xn", bufs=3,
                                 name=f"xn{snx}_{qt}")
                nc.sync.dma_start(
                    xn[:], x[snx * SEG + qt * 128: snx * SEG + (qt + 1) * 128, :])
                b0 = 2048
                _pp[0] += 2  # takes both slots
                for j in range(8):
                    nc.tensor.transpose(
                        ps[0:128, b0 + j * 128: b0 + (j + 1) * 128].bitcast(F32R),
                        xn[:, j * 128:(j + 1) * 128], idr[:])
                dst = xTall[:].rearrange("p (c w) -> p c w", w=512)[:, :, qt * 128:(qt + 1) * 128]
                src = ps[0:128, b0:b0 + 1024].rearrange("p (c w) -> p c w", w=128)
                nc.vector.tensor_copy(dst, src)

            def emit_Bdma(s, grp):
                """grp 0-3: QK (g=grp//2, half=grp%2); grp 4-5: V (vn=grp-4).
                All six are 8 DMAs of [128,512] w_attn column panels."""
                off = grp * 512
                tiles = []
                for cc in range(8):
                    w = stream.tile([128, 512], F32R, tag="ws", bufs=20,
                                    name=f"w{s}_{grp}_{cc}")
                    nc.sync.dma_start(
                        w[:], wa[cc * 128:(cc + 1) * 128, off: off + 512])
                    tiles.append(w)
                return tiles

            def emit_Bqk_unit(tiles, g, half, m):
                b = bank()
                for cc in range(8):
                    nc.tensor.matmul(
                        b, tiles[cc][:, m * 128:(m + 1) * 128],
                        xTall[:, cc * 512:(cc + 1) * 512],
                        start=(cc == 0), stop=(cc == 7))
                idx = g * 8 + half * 4 + m
                nc.vector.tensor_copy(qktall[:, idx * 512:(idx + 1) * 512], b)

            def emit_Bv_unit(tiles, vn, qt, vptiles):
                b = bank()
                for cc in range(8):
                    nc.tensor.matmul(
                        b, xTall[:, cc * 512 + qt * 128: cc * 512 + (qt + 1) * 128],
                        tiles[cc][:], start=(cc == 0), stop=(cc == 7))
                nc.vector.tensor_copy(
                    vptiles[qt].rearrange("p (h w) -> p h w", w=66)[:, vn * 8:(vn + 1) * 8, 0:64],
                    b.rearrange("p (h w) -> p h w", w=64))

            def emit_head(s, h, vptiles, Ytiles):
                q0 = (h // 2) * 512
                k0 = (8 + h // 2) * 512
                r0 = (h % 2) * 64
                at = stream.tile([128, 2048], BF16, tag="at0", bufs=3,
                                 name=f"at{s}_{h}")
                for half in range(2):
                    for k2 in range(2):
                        kt = half * 2 + k2
                        nc.tensor.matmul(
                            ps[0:128, kt * 512:(kt + 1) * 512],
                            qktall[r0:r0 + 64, k0 + kt * 128: k0 + (kt + 1) * 128],
                            qktall[r0:r0 + 64, q0:q0 + 512],
                            start=True, stop=True)
                    nc.scalar.activation(
                        at[:, half * 1024:(half + 1) * 1024],
                        ps[0:128, half * 1024:(half + 1) * 1024], EXP, scale=SCALE)
                base = 3072 + (h % 2) * 512
                for qt in range(4):
                    for kt in range(4):
                        nc.tensor.matmul(
                            ps[0:128, base + qt * 65: base + qt * 65 + 65],
                            at[:, kt * 512 + qt * 128: kt * 512 + (qt + 1) * 128],
                            vptiles[kt][:, 66 * h: 66 * h + 65],
                            start=(kt == 0), stop=(kt == 3))
                rz = stream.tile([128, 4], F32, tag="rz", bufs=2, name=f"rz{s}_{h}")
                nc.vector.reciprocal(
                    rz[:],
                    ps[0:128, base:base + 260].rearrange("p (a b) -> p a b", b=65)[:, :, 64:65])
                for qt in range(4):
                    nc.vector.tensor_scalar_mul(
                        Ytiles[qt][:, h * 64:(h + 1) * 64],
                        ps[0:128, base + qt * 65: base + qt * 65 + 64],
                        rz[:, qt:qt + 1])

            def emit_Yt(qt, Yprev, eng=None):
                b = bank()
                psb = b.bitcast(BF16)  # [128, 1024] bf16 = one f32 bank
                for cc in range(8):
                    nc.tensor.transpose(
                        psb[:, cc * 128:(cc + 1) * 128],
                        Yprev[qt][:, cc * 128:(cc + 1) * 128], idb[:])
                dst = ytall[:].rearrange("p (c w) -> p c w", w=512)[:, :, qt * 128:(qt + 1) * 128]
                src = psb.rearrange("p (c w) -> p c w", w=128)
                if eng == "scalar":
                    nc.scalar.copy(dst, src)
                else:
                    nc.vector.tensor_copy(dst, src)

            def emit_D(sp, m, vn, obt):
                b = bank()
                for cc in range(8):
                    nc.tensor.matmul(
                        b, ytall[:, cc * 512 + m * 128: cc * 512 + (m + 1) * 128],
                        wproj[cc][:, vn * 512:(vn + 1) * 512],
                        start=(cc == 0), stop=(cc == 7))
                nc.scalar.copy(obt[:, vn * 512:(vn + 1) * 512], b)
                nc.sync.dma_start(
                    out[sp * SEG + m * 128: sp * SEG + (m + 1) * 128,
                        vn * 512:(vn + 1) * 512],
                    obt[:, vn * 512:(vn + 1) * 512])

            # ---------------- prologue ----------------
            for qt in range(4):
                emit_A(0, qt)

            Yprev = None
            # first weight-DMA group of segment 0
            next_tiles = emit_Bdma(0, 0)
            for s in range(NSEG):
                vptiles = [stream.tile([128, 16 * 66], BF16, tag=f"vp{qt}",
                                       bufs=2, name=f"vp{s}_{qt}")
                           for qt in range(4)]
                for qt in range(4):
                    nc.vector.memset(
                        vptiles[qt].rearrange("p (h w) -> p h w", w=66)[:, :, 64:65], 1.0)

                # ---- B: 6 groups, DMA emitted one group ahead of use
                for grp in range(6):
                    tiles = next_tiles
                    if grp < 5:
                        next_tiles = emit_Bdma(s, grp + 1)
                    if grp < 4:
                        g, half = grp // 2, grp % 2
                        for m in range(4):
                            emit_Bqk_unit(tiles, g, half, m)
                    else:
                        vn = grp - 4
                        for qt in range(4):
                            emit_Bv_unit(tiles, vn, qt, vptiles)

                # ---- C: heads with pipelined fillers
                Ytiles = [stream.tile([128, C], BF16, tag=f"Y{qt}", bufs=2,
                                      name=f"Y{s}_{qt}")
                          for qt in range(4)]
                obtiles = [stream.tile([128, C], F32, tag="ob", bufs=2,
                                       name=f"ob{s}_{m}") for m in range(4)]
                fillers = []
                nonlocal_box = [None]
                if s < NSEG - 1:
                    # prefetch next segment's first weight group early
                    def _prefetch(snx):
                        nonlocal_box[0] = emit_Bdma(snx, 0)
                    fillers.append(lambda snx=s + 1: _prefetch(snx))
                    # x^T for next segment early: its producers retire as
                    # soon as heads start, and B_{s+1} needs it immediately.
                    for qt in range(4):
                        fillers.append(lambda qt=qt: emit_A(s + 1, qt))
                if s > 0:
                    for m in range(4):
                        fillers.append(lambda m=m: emit_Yt(m, Yprev))
                    for m in range(4):
                        fillers.append(lambda m=m: emit_D(s - 1, m, 0, obtiles[m]))
                        fillers.append(lambda m=m: emit_D(s - 1, m, 1, obtiles[m]))
                else:
                    def _wpstage(cc):
                        wst = stream.tile([128, C], F32R, tag="xn", bufs=3,
                                          name=f"wpst{cc}")
                        nc.sync.dma_start(wst[:], wp[cc * 128:(cc + 1) * 128, :])
                        nc.gpsimd.tensor_copy(wproj[cc][:], wst[:])
                    for cc in range(8):
                        fillers.append(lambda cc=cc: _wpstage(cc))
                nf = len(fillers)
                done = 0
                for h in range(16):
                    emit_head(s, h, vptiles, Ytiles)
                    want = (h + 1) * nf // 16
                    while done < want:
                        fillers[done]()
                        done += 1
                if s < NSEG - 1:
                    next_tiles = nonlocal_box[0]
                Yprev = Ytiles

            # ---- epilogue: last segment's projection (Yt first, then D)
            obtiles = [stream.tile([128, C], F32, tag="ob", bufs=2,
                                   name=f"obE_{m}") for m in range(4)]
            for m in range(4):
                emit_Yt(m, Yprev)
            for m in range(4):
                emit_D(NSEG - 1, m, 0, obtiles[m])
                emit_D(NSEG - 1, m, 1, obtiles[m])

    _split_multi_waits(nc)
    return nc


_NC = None


def kernel(x, w_attn, w_proj, split_sections):
    global _NC
    if _NC is None:
        _NC = _build()
    x = np.ascontiguousarray(np.asarray(x, dtype=np.float32))
    w_attn = np.ascontiguousarray(np.asarray(w_attn, dtype=np.float32))
    w_proj = np.ascontiguousarray(np.asarray(w_proj, dtype=np.float32))
    in_maps = [
        {"x_sh": x[i * TOK:(i + 1) * TOK], "w_attn": w_attn, "w_proj": w_proj}
        for i in range(NCORES)
    ]
    res = run_bass_kernel_spmd(_NC, in_maps, core_ids=list(range(NCORES)))
    return np.concatenate([res.results[i]["out"] for i in range(NCORES)], axis=0)


if __name__ == "__main__":
    rng = np.random.default_rng(0)
    x = rng.standard_normal((T, C), dtype=np.float32)
    wa = (rng.standard_normal((C, 3 * C), dtype=np.float32) / np.sqrt(C)).astype(np.float32)
    wpj = (rng.standard_normal((C, C), dtype=np.float32) / np.sqrt(C)).astype(np.float32)
    y = kernel(x, wa, wpj, np.arange(1, 32) * 512)
    print("out", y.shape, y.dtype, np.abs(y).mean())


# revision 18
# speedup vs baseline: 1.0422x; 1.0156x over previous
"""MHSA over 32 independent 512-token segments, segment-parallel across 8
NeuronCores (4 segments / 2048 tokens per core, zero cross-core traffic).

v2: software-pipelined single-pass design.

Per core, per segment s (tokens [512s, 512s+512)):
  x^T        via PE f32r transposes (identity moving operand, 1.5 cyc/row)
  Q^T,K^T    lhsT = streamed W tile (f32r straight from DMA; the DRAM
             tensors are declared float32r so no conversion copies), rhs =
             x^T chunks; accumulate per 128-col m-tile in one PSUM bank.
  V          natural [tok, 1024] via lhsT = x^T chunk, rhs = W tile -> bf16
  S^T        = K Q^T per head, 4 banks; exp via Act in 2 halves (bf16 out)
  O          natural per (head, qt): lhsT = A^T chunk [128k,128q] bf16,
             rhs = V|1 [128k, 65] bf16 -> PSUM [128q, 65]; 27 ns/matmul.
  normalize  reciprocal of col 64 (strided) + tensor_scalar_mul with the
             [128,1] per-partition scalar -> Y bf16 (no broadcast matmuls)
  Y^T        bf16 PE transposes (1 cyc/row) into a bf16-viewed PSUM bank
  out        = Y^T.T @ Wproj (lhsT bf16, rhs f32r)

Emission is pipelined: during segment s's attention heads (Act-bound), PE
is fed filler units: Y^T + projection of segment s-1 and x^T of segment
s+1. PE stays near 100% busy and never drops out of its high p-state.
"""

import numpy as np

import concourse.bass as bass
import concourse.mybir as mybir
import concourse.tile as tile
from concourse.bass_utils import run_bass_kernel_spmd

F32 = mybir.dt.float32
F32R = mybir.dt.float32r
BF16 = mybir.dt.bfloat16
EXP = mybir.ActivationFunctionType.Exp

T, C, H, HD = 16384, 1024, 16, 64
NCORES = 8
TOK = T // NCORES          # 2048 tokens per core
SEG = 512                  # tokens per segment
NSEG = TOK // SEG          # 4 segments per core
SCALE = 1.0 / np.sqrt(HD)  # folded into exp()


def _split_multi_waits(nc):
    """Move extra sync waits onto same-engine NoOps (1-wait ISA limit)."""
    for fn in nc.m.functions:
        for bb in fn.blocks:
            out = []
            for inst in bb.instructions:
                si = inst.sync_info
                if si is not None and si.on_wait and len(si.on_wait) > 1:
                    waits = list(si.on_wait)
                    for j, w in enumerate(waits[:-1]):
                        nop = mybir.InstNoOp(name=f"{inst.name}-wsp{j}")
                        nop.engine = inst.engine
                        nop.sync_info = mybir.SyncInfo(on_wait=[w], on_update=[])
                        out.append(nop)
                    inst.sync_info = mybir.SyncInfo(
                        on_wait=[waits[-1]], on_update=list(si.on_update)
                    )
                out.append(inst)
            bb.instructions = out


def _build():
    nc = bass.Bass("TRN2", target_bir_lowering=False, debug=False)
    x = nc.dram_tensor("x_sh", [TOK, C], F32R, kind="ExternalInput").ap()
    wa = nc.dram_tensor("w_attn", [C, 3 * C], F32R, kind="ExternalInput").ap()
    wp = nc.dram_tensor("w_proj", [C, C], F32R, kind="ExternalInput").ap()
    out = nc.dram_tensor("out", [TOK, C], F32, kind="ExternalOutput").ap()

    ident_d = nc.inline_tensor(np.eye(128, dtype=np.float32), "ident_c").ap()

    with tile.TileContext(nc) as tc:
        with (
            tc.tile_pool(name="const", bufs=1) as cpool,
            tc.tile_pool(name="wres", bufs=1) as wres,
            tc.tile_pool(name="stream", bufs=1) as stream,
            tc.tile_pool(name="work", bufs=1) as work,
            tc.tile_pool(name="ps", bufs=1, space="PSUM") as pspool,
        ):
            ps = pspool.tile([128, 4096], F32, tag="ps", name="ps")

            idf = cpool.tile([128, 128], F32, tag="idf", name="idf")
            idr = cpool.tile([128, 128], F32R, tag="idr", name="idr")
            idb = cpool.tile([128, 128], BF16, tag="idb", name="idb")
            nc.sync.dma_start(idf[:], ident_d[:, :])
            nc.vector.tensor_copy(idr[:], idf[:])
            nc.vector.tensor_copy(idb[:], idf[:])

            # wproj in bf16: walrus forbids mixing f32r with bf16 matmul
            # operands, and the Y^T side is bf16.
            wproj = [wres.tile([128, C], BF16, tag=f"wp{cc}", name=f"wp{cc}")
                     for cc in range(8)]

            # persistent single-generation big tiles
            xTall = work.tile([128, 4096], F32R, tag="xTall", name="xTall")
            qktall = work.tile([128, 8192], F32R, tag="qktall", name="qktall")
            ytall = work.tile([128, 4096], BF16, tag="ytall", name="ytall")

            # accumulation slot ping-pong over PSUM banks 4,5
            _pp = [0]

            def bank():
                b = 2048 + (_pp[0] % 2) * 512
                _pp[0] += 1
                return ps[0:128, b:b + 512]

            # ---------------- unit emitters ----------------
            def emit_A(snx, qt):
                """x^T for segment snx, token-tile qt -> xTall cols qt*128.
                Two bank-sized halves so consecutive units ping-pong PSUM."""
                xn = stream.tile([128, C], F32R, tag="xn", bufs=3,
                                 name=f"xn{snx}_{qt}")
                nc.sync.dma_start(
                    xn[:], x[snx * SEG + qt * 128: snx * SEG + (qt + 1) * 128, :])
                for hf in range(2):
                    b = bank()
                    for j in range(4):
                        cc = hf * 4 + j
                        nc.tensor.transpose(
                            b[:, j * 128:(j + 1) * 128].bitcast(F32R),
                            xn[:, cc * 128:(cc + 1) * 128], idr[:])
                    dst = (xTall[:, hf * 2048:(hf + 1) * 2048]
                           .rearrange("p (c w) -> p c w", w=512)[:, :, qt * 128:(qt + 1) * 128])
                    src = b.rearrange("p (c w) -> p c w", w=128)
                    nc.vector.tensor_copy(dst, src)

            def emit_Bdma(s, grp):
                """grp 0-3: QK (g=grp//2, half=grp%2); grp 4-5: V (vn=grp-4).
                All six are 8 DMAs of [128,512] w_attn column panels."""
                off = grp * 512
                tiles = []
                for cc in range(8):
                    w = stream.tile([128, 512], F32R, tag="ws", bufs=20,
                                    name=f"w{s}_{grp}_{cc}")
                    nc.sync.dma_start(
                        w[:], wa[cc * 128:(cc + 1) * 128, off: off + 512])
                    tiles.append(w)
                return tiles

            def emit_Bqk_unit(tiles, g, half, m):
                b = bank()
                for cc in range(8):
                    nc.tensor.matmul(
                        b, tiles[cc][:, m * 128:(m + 1) * 128],
                        xTall[:, cc * 512:(cc + 1) * 512],
                        start=(cc == 0), stop=(cc == 7))
                idx = g * 8 + half * 4 + m
                nc.vector.tensor_copy(qktall[:, idx * 512:(idx + 1) * 512], b)

            def emit_Bv_unit(tiles, vn, qt, vptiles):
                b = bank()
                for cc in range(8):
                    nc.tensor.matmul(
                        b, xTall[:, cc * 512 + qt * 128: cc * 512 + (qt + 1) * 128],
                        tiles[cc][:], start=(cc == 0), stop=(cc == 7))
                nc.vector.tensor_copy(
                    vptiles[qt].rearrange("p (h w) -> p h w", w=66)[:, vn * 8:(vn + 1) * 8, 0:64],
                    b.rearrange("p (h w) -> p h w", w=64))

            def emit_head(s, h, vptiles, Ytiles):
                q0 = (h // 2) * 512
                k0 = (8 + h // 2) * 512
                r0 = (h % 2) * 64
                at = stream.tile([128, 2048], BF16, tag="at0", bufs=3,
                                 name=f"at{s}_{h}")
                for half in range(2):
                    for k2 in range(2):
                        kt = half * 2 + k2
                        nc.tensor.matmul(
                            ps[0:128, kt * 512:(kt + 1) * 512],
                            qktall[r0:r0 + 64, k0 + kt * 128: k0 + (kt + 1) * 128],
                            qktall[r0:r0 + 64, q0:q0 + 512],
                            start=True, stop=True)
                    nc.scalar.activation(
                        at[:, half * 1024:(half + 1) * 1024],
                        ps[0:128, half * 1024:(half + 1) * 1024], EXP, scale=SCALE)
                base = 3072 + (h % 2) * 512
                for qt in range(4):
                    for kt in range(4):
                        nc.tensor.matmul(
                            ps[0:128, base + qt * 65: base + qt * 65 + 65],
                            at[:, kt * 512 + qt * 128: kt * 512 + (qt + 1) * 128],
                            vptiles[kt][:, 66 * h: 66 * h + 65],
                            start=(kt == 0), stop=(kt == 3))
                rz = stream.tile([128, 4], F32, tag="rz", bufs=2, name=f"rz{s}_{h}")
                nc.vector.reciprocal(
                    rz[:],
                    ps[0:128, base:base + 260].rearrange("p (a b) -> p a b", b=65)[:, :, 64:65])
                for qt in range(4):
                    nc.vector.tensor_scalar_mul(
                        Ytiles[qt][:, h * 64:(h + 1) * 64],
                        ps[0:128, base + qt * 65: base + qt * 65 + 64],
                        rz[:, qt:qt + 1])

            def emit_Yt(qt, Yprev, eng=None):
                b = bank()
                psb = b.bitcast(BF16)  # [128, 1024] bf16 = one f32 bank
                for cc in range(8):
                    nc.tensor.transpose(
                        psb[:, cc * 128:(cc + 1) * 128],
                        Yprev[qt][:, cc * 128:(cc + 1) * 128], idb[:])
                dst = ytall[:].rearrange("p (c w) -> p c w", w=512)[:, :, qt * 128:(qt + 1) * 128]
                src = psb.rearrange("p (c w) -> p c w", w=128)
                if eng == "scalar":
                    nc.scalar.copy(dst, src)
                else:
                    nc.vector.tensor_copy(dst, src)

            def emit_D(sp, m, vn, obt):
                b = bank()
                for cc in range(8):
                    nc.tensor.matmul(
                        b, ytall[:, cc * 512 + m * 128: cc * 512 + (m + 1) * 128],
                        wproj[cc][:, vn * 512:(vn + 1) * 512],
                        start=(cc == 0), stop=(cc == 7))
                nc.scalar.copy(obt[:, vn * 512:(vn + 1) * 512], b)
                nc.sync.dma_start(
                    out[sp * SEG + m * 128: sp * SEG + (m + 1) * 128,
                        vn * 512:(vn + 1) * 512],
                    obt[:, vn * 512:(vn + 1) * 512])

            # ---------------- prologue ----------------
            for qt in range(4):
                emit_A(0, qt)

            Yprev = None
            # first weight-DMA group of segment 0
            next_tiles = emit_Bdma(0, 0)
            for s in range(NSEG):
                vptiles = [stream.tile([128, 16 * 66], BF16, tag=f"vp{qt}",
                                       bufs=2, name=f"vp{s}_{qt}")
                           for qt in range(4)]
                for qt in range(4):
                    nc.vector.memset(
                        vptiles[qt].rearrange("p (h w) -> p h w", w=66)[:, :, 64:65], 1.0)

                # ---- B: 6 groups, DMA emitted one group ahead of use
                for grp in range(6):
                    tiles = next_tiles
                    if grp < 5:
                        next_tiles = emit_Bdma(s, grp + 1)
                    if grp < 4:
                        g, half = grp // 2, grp % 2
                        for m in range(4):
                            emit_Bqk_unit(tiles, g, half, m)
                    else:
                        vn = grp - 4
                        for qt in range(4):
                            emit_Bv_unit(tiles, vn, qt, vptiles)

                # ---- C: heads with pipelined fillers
                Ytiles = [stream.tile([128, C], BF16, tag=f"Y{qt}", bufs=2,
                                      name=f"Y{s}_{qt}")
                          for qt in range(4)]
                obtiles = [stream.tile([128, C], F32, tag="ob", bufs=2,
                                       name=f"ob{s}_{m}") for m in range(4)]
                fillers = []
                nonlocal_box = [None]
                if s < NSEG - 1:
                    # prefetch next segment's first weight group early
                    def _prefetch(snx):
                        nonlocal_box[0] = emit_Bdma(snx, 0)
                    fillers.append(lambda snx=s + 1: _prefetch(snx))
                    # x^T for next segment early: its producers retire as
                    # soon as heads start, and B_{s+1} needs it immediately.
                    for qt in range(4):
                        fillers.append(lambda qt=qt: emit_A(s + 1, qt))
                if s > 0:
                    for m in range(4):
                        fillers.append(lambda m=m: emit_Yt(m, Yprev))
                    for m in range(4):
                        fillers.append(lambda m=m: emit_D(s - 1, m, 0, obtiles[m]))
                        fillers.append(lambda m=m: emit_D(s - 1, m, 1, obtiles[m]))
                else:
                    def _wpstage(cc):
                        wst = stream.tile([128, C], F32R, tag="xn", bufs=3,
                                          name=f"wpst{cc}")
                        nc.sync.dma_start(wst[:], wp[cc * 128:(cc + 1) * 128, :])
                        nc.gpsimd.tensor_copy(wproj[cc][:], wst[:])
                    for cc in range(8):
                        fillers.append(lambda cc=cc: _wpstage(cc))
                nf = len(fillers)
                done = 0
                for h in range(16):
                    emit_head(s, h, vptiles, Ytiles)
                    want = min(nf, ((h + 1) * nf + 11) // 16)
                    while done < want:
                        fillers[done]()
                        done += 1
                if s < NSEG - 1:
                    next_tiles = nonlocal_box[0]
                Yprev = Ytiles

            # ---- epilogue: last segment's projection (Yt first, then D)
            obtiles = [stream.tile([128, C], F32, tag="ob", bufs=2,
                                   name=f"obE_{m}") for m in range(4)]
            for m in range(4):
                emit_Yt(m, Yprev)
            for m in range(4):
                emit_D(NSEG - 1, m, 0, obtiles[m])
                emit_D(NSEG - 1, m, 1, obtiles[m])

    _split_multi_waits(nc)
    return nc


_NC = None


def kernel(x, w_attn, w_proj, split_sections):
    global _NC
    if _NC is None:
        _NC = _build()
    x = np.ascontiguousarray(np.asarray(x, dtype=np.float32))
    w_attn = np.ascontiguousarray(np.asarray(w_attn, dtype=np.float32))
    w_proj = np.ascontiguousarray(np.asarray(w_proj, dtype=np.float32))
    in_maps = [
        {"x_sh": x[i * TOK:(i + 1) * TOK], "w_attn": w_attn, "w_proj": w_proj}
        for i in range(NCORES)
    ]
    res = run_bass_kernel_spmd(_NC, in_maps, core_ids=list(range(NCORES)))
    return np.concatenate([res.results[i]["out"] for i in range(NCORES)], axis=0)


if __name__ == "__main__":
    rng = np.random.default_rng(0)
    x = rng.standard_normal((T, C), dtype=np.float32)
    wa = (rng.standard_normal((C, 3 * C), dtype=np.float32) / np.sqrt(C)).astype(np.float32)
    wpj = (rng.standard_normal((C, C), dtype=np.float32) / np.sqrt(C)).astype(np.float32)
    y = kernel(x, wa, wpj, np.arange(1, 32) * 512)
    print("out", y.shape, y.dtype, np.abs(y).mean())
